# revision 4
# baseline (speedup 1.0000x reference)
"""CrossAttentionBlock on 8 Trainium2 NeuronCores.

Sharding: data-parallel over B=4 (one batch per core pair), tensor-parallel
rank t = core % 2 within the pair:
  - attention: heads split 8/8 (each core: kv, qh, scores, softmax, ctx for
    its 8 heads over all 1024 query rows; writes its head-shard of attn out)
  - proj: partial product with its 512-row slice of Wproj, pairwise
    ReduceScatter(add) leaves each core the summed y for its 512 query rows
  - MLP: row-split — full 4096-wide MLP for its 512 rows.

All matmuls in bf16 (fp32 PSUM accumulation); layernorm/softmax norm in fp32.
Softmax skips max-subtraction (logits bounded ~±3 for 0.02-scaled weights).
"""

import numpy as np
import ml_dtypes

import concourse.bass as bass
import concourse.mybir as mybir
import concourse.tile as tile
from concourse import bacc
from concourse.bass_utils import run_bass_kernel_spmd
from concourse.masks import make_identity

F32 = mybir.dt.float32
BF16 = mybir.dt.bfloat16
AF = mybir.ActivationFunctionType
ALU = mybir.AluOpType
AX = mybir.AxisListType

B, n, N, C, H = 4, 1024, 2048, 1024, 4096
NH, D = 16, 64          # heads, head dim
HL = NH // 2            # heads per core (8)
RT = n // 2             # rows per core for proj/MLP (512)
EPS = 1e-5
SCALE = D ** -0.5

_cache = {}


def _alt(i):
    return "vector" if i % 2 == 0 else "scalar"


def _copy(nc, eng, out, in_):
    if eng == "vector":
        nc.vector.tensor_copy(out=out, in_=in_)
    else:
        nc.scalar.copy(out=out, in_=in_)


def _ln_apply(nc, pool, src_ap, g_bc, bt_bc, eps_t, out_dtype, tagp):
    """LayerNorm a [128, C] fp32 SBUF AP; returns normalized tile."""
    st = pool.tile([128, 2, 6], F32, tag=tagp + "st")
    nc.vector.bn_stats(out=st[:, 0], in_=src_ap[:, 0:512])
    nc.vector.bn_stats(out=st[:, 1], in_=src_ap[:, 512:1024])
    mv = pool.tile([128, 2], F32, tag=tagp + "mv")
    nc.vector.bn_aggr(out=mv, in_=st)
    nc.scalar.activation(out=mv[:, 1:2], in_=mv[:, 1:2],
                         func=AF.Sqrt, bias=eps_t, scale=1.0)
    nc.vector.reciprocal(out=mv[:, 1:2], in_=mv[:, 1:2])
    t = pool.tile([128, C], F32, tag=tagp + "t")
    nc.vector.tensor_scalar(out=t, in0=src_ap,
                            scalar1=mv[:, 0:1], scalar2=mv[:, 1:2],
                            op0=ALU.subtract, op1=ALU.mult)
    nc.vector.tensor_mul(out=t, in0=t, in1=g_bc)
    o = pool.tile([128, C], out_dtype, tag=tagp + "o")
    nc.vector.tensor_add(out=o, in0=t, in1=bt_bc)
    return o


def _build():
    nc = bacc.Bacc(num_devices=8)

    qT_d = nc.declare_dram_parameter("qT", [C, n], BF16, isOutput=False)
    qrows_d = nc.declare_dram_parameter("qrows", [RT, C], F32, isOutput=False)
    x_d = nc.declare_dram_parameter("x", [N, C], F32, isOutput=False)
    wq_d = nc.declare_dram_parameter("wq", [C, HL * D], BF16, isOutput=False)
    wk_d = nc.declare_dram_parameter("wk", [C, HL * D], BF16, isOutput=False)
    wv_d = nc.declare_dram_parameter("wv", [C, HL * D], BF16, isOutput=False)
    wp_d = nc.declare_dram_parameter("wp", [HL * D, C], BF16, isOutput=False)
    w1_d = nc.declare_dram_parameter("w1", [C, H], BF16, isOutput=False)
    w2_d = nc.declare_dram_parameter("w2", [H, C], BF16, isOutput=False)
    bproj_d = nc.declare_dram_parameter("bproj", [C], F32, isOutput=False)
    b1_d = nc.declare_dram_parameter("b1", [H], F32, isOutput=False)
    b2_d = nc.declare_dram_parameter("b2", [C], F32, isOutput=False)
    g1_d = nc.declare_dram_parameter("g1", [C], F32, isOutput=False)
    bt1_d = nc.declare_dram_parameter("bt1", [C], F32, isOutput=False)
    g2_d = nc.declare_dram_parameter("g2", [C], F32, isOutput=False)
    bt2_d = nc.declare_dram_parameter("bt2", [C], F32, isOutput=False)

    qo_d = nc.declare_dram_parameter("qo", [RT, C], F32, isOutput=True)
    attn_d = nc.declare_dram_parameter("attn", [HL, n, N], F32, isOutput=True)

    ybounce = nc.dram_tensor("ybounce", [n, C], F32)
    yrs = nc.dram_tensor("yrs", [RT, C], F32)

    def bcast_row(vec_d, width):
        return bass.AP(tensor=vec_d.ap().tensor, offset=0,
                       ap=[[0, 128], [1, width]])

    with tile.TileContext(nc) as tc, \
         tc.tile_pool(name="const", bufs=1) as const, \
         tc.tile_pool(name="big", bufs=1) as big:

        ident_b = const.tile([128, 128], BF16)
        make_identity(nc, ident_b)
        ident_f = const.tile([128, 128], F32)
        make_identity(nc, ident_f)
        eps_t = const.tile([128, 1], F32)
        nc.vector.memset(eps_t, EPS)
        g1_bc = const.tile([128, C], F32)
        nc.sync.dma_start(out=g1_bc, in_=bcast_row(g1_d, C))
        bt1_bc = const.tile([128, C], F32)
        nc.sync.dma_start(out=bt1_bc, in_=bcast_row(bt1_d, C))
        g2_bc = const.tile([128, C], F32)
        nc.sync.dma_start(out=g2_bc, in_=bcast_row(g2_d, C))
        bt2_bc = const.tile([128, C], F32)
        nc.sync.dma_start(out=bt2_bc, in_=bcast_row(bt2_d, C))
        bproj_bc = const.tile([128, C], F32)
        nc.sync.dma_start(out=bproj_bc, in_=bcast_row(bproj_d, C))
        b2_bc = const.tile([128, C], F32)
        nc.sync.dma_start(out=b2_bc, in_=bcast_row(b2_d, C))
        b1_col = const.tile([128, H // 128], F32)
        nc.sync.dma_start(out=b1_col,
                          in_=b1_d.ap().rearrange("(m p) -> p m", p=128))
        wp_sb = const.tile([128, 4, C], BF16)
        nc.sync.dma_start(out=wp_sb,
                          in_=wp_d.ap().rearrange("(c p) m -> p c m", p=128))

        # whole-kernel activations (48 KB/partition)
        kT_sb = big.tile([128, 4, N], BF16)       # k^T  [512, 2048]
        v_sb = big.tile([128, 16, HL * D], BF16)  # v    [2048, 512]
        qhT_sb = big.tile([128, 4, n], BF16)      # qh^T [512, 1024]
        ctxT_sb = big.tile([128, 4, n], BF16)     # ctx^T[512, 1024]

        # ---- stages 1-2: LN(x) -> xn^T ; k^T, v, qh^T --------------------
        with tc.tile_pool(name="s12", bufs=1) as s12, \
             tc.tile_pool(name="s1w", bufs=3) as s1w, \
             tc.tile_pool(name="s1p", bufs=4, space="PSUM") as s1p:
            qT_sb = s12.tile([128, 8, n], BF16)
            nc.sync.dma_start(out=qT_sb,
                              in_=qT_d.ap().rearrange("(c p) m -> p c m", p=128))
            wq_sb = s12.tile([128, 8, HL * D], BF16)
            nc.sync.dma_start(out=wq_sb,
                              in_=wq_d.ap().rearrange("(c p) m -> p c m", p=128))
            wk_sb = s12.tile([128, 8, HL * D], BF16)
            nc.sync.dma_start(out=wk_sb,
                              in_=wk_d.ap().rearrange("(c p) m -> p c m", p=128))
            wv_sb = s12.tile([128, 8, HL * D], BF16)
            nc.sync.dma_start(out=wv_sb,
                              in_=wv_d.ap().rearrange("(c p) m -> p c m", p=128))
            xnT_sb = s12.tile([128, 8, N], BF16)  # xn^T [1024, 2048]

            for rt in range(16):
                xt = s1w.tile([128, C], F32, tag="x")
                nc.sync.dma_start(out=xt,
                                  in_=x_d.ap()[rt * 128:(rt + 1) * 128, :])
                xo = _ln_apply(nc, s1w, xt, g1_bc, bt1_bc, eps_t, BF16, "l1")
                for cc in range(8):
                    tp = s1p.tile([128, 128], BF16, tag="tp")
                    nc.tensor.transpose(tp, xo[:, cc * 128:(cc + 1) * 128], ident_b)
                    _copy(nc, _alt(cc), xnT_sb[:, cc, rt * 128:(rt + 1) * 128], tp)

            with tc.tile_pool(name="s2p", bufs=4, space="PSUM") as s2p:
                for mt in range(4):
                    for nch in range(4):
                        ps = s2p.tile([128, 512], F32, tag="mm")
                        for kc in range(8):
                            nc.tensor.matmul(
                                ps, wk_sb[:, kc, mt * 128:(mt + 1) * 128],
                                xnT_sb[:, kc, nch * 512:(nch + 1) * 512],
                                start=(kc == 0), stop=(kc == 7))
                        _copy(nc, _alt(mt + nch),
                              kT_sb[:, mt, nch * 512:(nch + 1) * 512], ps)
                for nt in range(16):
                    ps = s2p.tile([128, 512], F32, tag="mm")
                    for kc in range(8):
                        nc.tensor.matmul(
                            ps, xnT_sb[:, kc, nt * 128:(nt + 1) * 128],
                            wv_sb[:, kc, :], start=(kc == 0), stop=(kc == 7))
                    _copy(nc, _alt(nt), v_sb[:, nt, :], ps)
                for mt in range(4):
                    for nch in range(2):
                        ps = s2p.tile([128, 512], F32, tag="mm")
                        for kc in range(8):
                            nc.tensor.matmul(
                                ps, wq_sb[:, kc, mt * 128:(mt + 1) * 128],
                                qT_sb[:, kc, nch * 512:(nch + 1) * 512],
                                start=(kc == 0), stop=(kc == 7))
                        _copy(nc, _alt(mt + nch),
                              qhT_sb[:, mt, nch * 512:(nch + 1) * 512], ps)

        # ---- stage 3: attention per head --------------------------------
        with tc.tile_pool(name="att", bufs=2) as att, \
             tc.tile_pool(name="atts", bufs=2) as atts, \
             tc.tile_pool(name="ptp", bufs=1) as ptp, \
             tc.tile_pool(name="attp", bufs=2, space="PSUM") as attp, \
             tc.tile_pool(name="ctxp", bufs=2, space="PSUM") as ctxp:
            for hl in range(HL):
                toff = hl // 2
                poff = (hl % 2) * 64
                PT = ptp.tile([128, 16, n], BF16, tag="PT")    # P^T [2048, 1024]
                recT = ptp.tile([1, 8, 128], F32, tag="recT")  # 1/rowsum [1, 1024]
                for m in range(8):
                    P_sb = att.tile([128, 4, 512], BF16, tag="P")
                    rs4 = att.tile([128, 4], F32, tag="rs4")
                    for c in range(4):
                        sc = attp.tile([128, 512], F32, tag="sc")
                        nc.tensor.matmul(
                            sc,
                            qhT_sb[poff:poff + 64, toff, m * 128:(m + 1) * 128],
                            kT_sb[poff:poff + 64, toff, c * 512:(c + 1) * 512],
                            start=True, stop=True)
                        nc.scalar.activation(out=P_sb[:, c, :], in_=sc,
                                             func=AF.Exp, scale=SCALE,
                                             accum_out=rs4[:, c:c + 1])
                    rsum = att.tile([128, 1], F32, tag="rsum")
                    nc.vector.tensor_reduce(out=rsum, in_=rs4, axis=AX.X, op=ALU.add)
                    rec = att.tile([128, 1], F32, tag="rec")
                    nc.vector.reciprocal(out=rec, in_=rsum)
                    ao = atts.tile([128, 4, 512], F32, tag="ao")
                    nc.vector.tensor_scalar_mul(out=ao, in0=P_sb, scalar1=rec)
                    nc.sync.dma_start(
                        out=attn_d.ap()[hl, m * 128:(m + 1) * 128, :], in_=ao)
                    rtp = attp.tile([1, 128], F32, tag="rtp")
                    nc.tensor.transpose(rtp, rec, ident_f)
                    nc.vector.tensor_copy(out=recT[0:1, m, :], in_=rtp)
                    for c in range(4):
                        tp = attp.tile([128, 4, 128], BF16, tag="tp")
                        for bi in range(4):
                            nc.tensor.transpose(
                                tp[:, bi, :],
                                P_sb[:, c, bi * 128:(bi + 1) * 128], ident_b)
                        _copy(nc, _alt(c),
                              PT[:, 4 * c:4 * c + 4, m * 128:(m + 1) * 128], tp)
                rbc = att.tile([64, n], F32, tag="rbc")
                nc.gpsimd.partition_broadcast(rbc, recT[0:1, :, :])
                for rch in range(2):
                    cps = ctxp.tile([64, 512], F32, tag="cps")
                    for nt in range(16):
                        nc.tensor.matmul(
                            cps, v_sb[:, nt, hl * 64:(hl + 1) * 64],
                            PT[:, nt, rch * 512:(rch + 1) * 512],
                            start=(nt == 0), stop=(nt == 15))
                    nc.vector.tensor_mul(
                        out=ctxT_sb[poff:poff + 64, toff,
                                    rch * 512:(rch + 1) * 512],
                        in0=cps, in1=rbc[:, rch * 512:(rch + 1) * 512])

        # ---- stage 4: proj partial + ReduceScatter ----------------------
        with tc.tile_pool(name="prj", bufs=3) as prj, \
             tc.tile_pool(name="prjp", bufs=4, space="PSUM") as prjp:
            for m in range(8):
                for nch in range(2):
                    ps = prjp.tile([128, 512], F32, tag="y")
                    for kc in range(4):
                        nc.tensor.matmul(
                            ps, ctxT_sb[:, kc, m * 128:(m + 1) * 128],
                            wp_sb[:, kc, nch * 512:(nch + 1) * 512],
                            start=(kc == 0), stop=(kc == 3))
                    yt = prj.tile([128, 512], F32, tag="yt")
                    _copy(nc, _alt(m + nch), yt, ps)
                    nc.sync.dma_start(
                        out=ybounce.ap()[m * 128:(m + 1) * 128,
                                         nch * 512:(nch + 1) * 512],
                        in_=yt)
            nc.gpsimd.collective_compute(
                "ReduceScatter", ALU.add,
                replica_groups=[[0, 1], [2, 3], [4, 5], [6, 7]],
                ins=[ybounce.ap()], outs=[yrs.ap()])

        # ---- stages 5-6: residual + LN2 + MLP ---------------------------
        with tc.tile_pool(name="mlp", bufs=1) as mlp:
            q2_sb = mlp.tile([128, 4, C], F32)
            xqT_sb = mlp.tile([128, 8, RT], BF16)
            hidT_sb = mlp.tile([128, 32, RT], BF16)

            with tc.tile_pool(name="s5", bufs=3) as s5, \
                 tc.tile_pool(name="s5p", bufs=4, space="PSUM") as s5p:
                for m in range(4):
                    yt = s5.tile([128, C], F32, tag="yt")
                    nc.sync.dma_start(out=yt,
                                      in_=yrs.ap()[m * 128:(m + 1) * 128, :])
                    qt = s5.tile([128, C], F32, tag="qt")
                    nc.sync.dma_start(out=qt,
                                      in_=qrows_d.ap()[m * 128:(m + 1) * 128, :])
                    nc.vector.tensor_add(out=yt, in0=yt, in1=bproj_bc)
                    nc.vector.tensor_add(out=q2_sb[:, m, :], in0=yt, in1=qt)
                for m in range(4):
                    xqb = _ln_apply(nc, s5, q2_sb[:, m, :], g2_bc, bt2_bc,
                                    eps_t, BF16, "l2")
                    for cc in range(8):
                        tp = s5p.tile([128, 128], BF16, tag="tp")
                        nc.tensor.transpose(tp, xqb[:, cc * 128:(cc + 1) * 128],
                                            ident_b)
                        _copy(nc, _alt(cc),
                              xqT_sb[:, cc, m * 128:(m + 1) * 128], tp)

            with tc.tile_pool(name="w1s", bufs=2) as w1s, \
                 tc.tile_pool(name="m1p", bufs=4, space="PSUM") as m1p:
                for g in range(8):
                    w1t = w1s.tile([128, 8, 512], BF16, tag="w1")
                    nc.sync.dma_start(
                        out=w1t,
                        in_=w1_d.ap()[:, g * 512:(g + 1) * 512]
                        .rearrange("(c p) m -> p c m", p=128))
                    for i in range(4):
                        hm = g * 4 + i
                        ps = m1p.tile([128, 512], F32, tag="hid")
                        for kc in range(8):
                            nc.tensor.matmul(
                                ps, w1t[:, kc, i * 128:(i + 1) * 128],
                                xqT_sb[:, kc, :], start=(kc == 0), stop=(kc == 7))
                        nc.scalar.activation(out=hidT_sb[:, hm, :], in_=ps,
                                             func=AF.Gelu,
                                             bias=b1_col[:, hm:hm + 1], scale=1.0)

            with tc.tile_pool(name="w2s", bufs=3) as w2s, \
                 tc.tile_pool(name="m2p", bufs=1, space="PSUM") as m2p, \
                 tc.tile_pool(name="outp", bufs=3) as outp:
                y2ps = [[m2p.tile([128, 512], F32, tag=f"y2_{m}_{c}",
                                  name=f"y2ps_{m}_{c}")
                         for c in range(2)] for m in range(4)]
                for hk in range(32):
                    w2t = w2s.tile([128, C], BF16, tag="w2")
                    nc.sync.dma_start(out=w2t,
                                      in_=w2_d.ap()[hk * 128:(hk + 1) * 128, :])
                    for m in range(4):
                        for c in range(2):
                            nc.tensor.matmul(
                                y2ps[m][c],
                                hidT_sb[:, hk, m * 128:(m + 1) * 128],
                                w2t[:, c * 512:(c + 1) * 512],
                                start=(hk == 0), stop=(hk == 31))
                for m in range(4):
                    for c in range(2):
                        ot = outp.tile([128, 512], F32, tag="ot")
                        nc.vector.tensor_add(
                            out=ot, in0=y2ps[m][c],
                            in1=q2_sb[:, m, c * 512:(c + 1) * 512])
                        nc.vector.tensor_add(out=ot, in0=ot,
                                             in1=b2_bc[:, c * 512:(c + 1) * 512])
                        nc.sync.dma_start(
                            out=qo_d.ap()[m * 128:(m + 1) * 128,
                                          c * 512:(c + 1) * 512],
                            in_=ot)

    nc.finalize()
    return nc


def _get_program():
    if "nc" not in _cache:
        _cache["nc"] = _build()
    return _cache["nc"]


def _make_in_maps(q, x, Wq, Wkv, Wproj, bproj, W1, b1, W2, b2,
                  g1, beta1, g2, beta2):
    q = np.asarray(q, np.float32)
    x = np.asarray(x, np.float32)
    bf = lambda a: np.ascontiguousarray(np.asarray(a, np.float32)).astype(
        ml_dtypes.bfloat16)
    f32 = lambda a: np.ascontiguousarray(np.asarray(a, np.float32))

    Wq_b, Wkv_b, Wp_b = bf(Wq), bf(Wkv), bf(Wproj)
    W1_b, W2_b = bf(W1), bf(W2)

    in_maps = []
    for core in range(8):
        b, t = core // 2, core % 2
        hs = t * HL * D
        in_maps.append({
            "qT": np.ascontiguousarray(bf(q[b]).T),
            "qrows": f32(q[b, t * RT:(t + 1) * RT]),
            "x": f32(x[b]),
            "wq": np.ascontiguousarray(Wq_b[:, hs:hs + HL * D]),
            "wk": np.ascontiguousarray(Wkv_b[:, hs:hs + HL * D]),
            "wv": np.ascontiguousarray(Wkv_b[:, C + hs:C + hs + HL * D]),
            "wp": np.ascontiguousarray(Wp_b[hs:hs + HL * D, :]),
            "w1": W1_b,
            "w2": W2_b,
            "bproj": f32(bproj), "b1": f32(b1), "b2": f32(b2),
            "g1": f32(g1), "bt1": f32(beta1), "g2": f32(g2), "bt2": f32(beta2),
        })
    return in_maps


def _gather(res):
    q_out = np.empty((B, n, C), np.float32)
    attn = np.empty((B, NH, n, N), np.float32)
    for core in range(8):
        b, t = core // 2, core % 2
        r = res.results[core]
        q_out[b, t * RT:(t + 1) * RT] = r["qo"]
        attn[b, t * HL:(t + 1) * HL] = r["attn"]
    return (q_out, attn)


def kernel(q, x, Wq, Wkv, Wproj, bproj, W1, b1, W2, b2, g1, beta1, g2, beta2):
    in_maps = _make_in_maps(q, x, Wq, Wkv, Wproj, bproj, W1, b1, W2, b2,
                            g1, beta1, g2, beta2)
    nc = _get_program()
    res = run_bass_kernel_spmd(nc, in_maps, core_ids=list(range(8)))
    return _gather(res)


def run_profiled(inputs):
    """Run with NTFF tracing; returns exec_time_ns (requires hook shim)."""
    in_maps = _make_in_maps(**inputs)
    nc = _get_program()
    res = run_bass_kernel_spmd(nc, in_maps, core_ids=list(range(8)), trace=True)
    _cache["last_results"] = res
    return res.exec_time_ns
